# revision 34
# baseline (speedup 1.0000x reference)
"""Correlation layer (FlowNet-style) Trainium2 Bass kernel.

Problem: in1, in2: [8, 256, 128, 128] fp32.
out[b, 9*dy+dx, y, x] = mean_c in1[b,c,y,x] * in2pad[b,c,y+dy,x+dx],
with in2 zero-padded by 4 on each spatial side, dy,dx in [0,9).
Output: [8, 81, 128, 128] fp32.

Sharding: data-parallel over batch -> 8 NeuronCores, one batch each
(SPMD: identical program, per-core input slices).

Per-core algorithm (all-bf16 datapath; 1/256 mean folded into in1 on host):
  Phase 1 (Gram matmuls), tiles of 128 output pixels (y-block 32 x x-block 4):
      stationary = in1[c, ytile, xtile]  (128 cols, x-outer/y-inner:
                                          i = x_off*32 + y_off)
      moving     = in2pad[c, y0:y0+40, x0:x0+12]  (480 cols)
      psum[i, j] = sum_c stat[c,i] * mov[c,j]  (accumulated over 2 c-blocks)
    The 81 correlation outputs of pixel i sit at j = (y_off+dy)*12+(x_off+dx),
    a sheared band.  Window-compact straight out of PSUM (cast fp32->bf16)
    per 32-partition group g (all pixels of a group share x_off=g, so the
    9-col window [g, g+9) is partition-uniform).  In the compacted [40, 9]
    block the 81 useful values of pixel (g, u) are one contiguous run at
    offset 9u.  Batches of KB=8 tiles dump to a per-batch DRAM scratch
    (one DMA per batch).
  Phase 2 per batch: one 4D-AP DMA gather (flat DRAM addressing absorbs the
    partition-dependent run offset 9u), TensorE transpose
    [pixel, 81] -> [81, pixel] per tile, evacuate with the
    (x-outer,y-inner) -> (y,x) reorder into a per-yb row-block, store
    [81, 32, 128] bf16 row-blocks (host upcasts to fp32).
"""

import numpy as np
from contextlib import ExitStack

import ml_dtypes

import concourse.bacc as bacc
import concourse.tile as tile
import concourse.mybir as mybir
import concourse.bass as bass
from concourse import bass_utils

# ---- problem constants (hardcoded per contract) ----
B = 8
C = 256
H = W = 128
PAD = 4
D = 9            # displacements per axis
CH = D * D       # 81 output channels
HP = WP = H + 2 * PAD   # 136 padded

YB = 32          # y rows per tile
XBW = 4          # x cols per tile (stationary width)
MV_Y = YB + 8    # moving window rows   (40)
MV_X = XBW + 8   # moving window cols   (12)
N_YB = H // YB   # 4
N_XB = W // XBW  # 32
N_TILES = N_YB * N_XB   # 128
NG = 128 // YB   # 4 groups of 32 partitions per tile
WIN = MV_Y * D   # 360 compacted window elems per pixel

KB = 16                  # tiles per scratch batch (last y-block tapers)
# per-y-block batch sizes: taper the final y-block so the drain tail
# (last dump -> gather -> transpose -> out) is short
YB_BATCHES = [[16, 16], [16, 16], [16, 16], [16, 8, 4, 4]]
NBATCH = sum(len(b) for b in YB_BATCHES)
# batches whose de-shear runs as a gpsimd local_scatter (SBUF-only, no DRAM
# bounce, no compacts) instead of the dump->gather path; scattering the later
# batches drains the tail where the DMA engines idle anyway
SCATTER = [False, True, False, True, False, True, False, True, True, True]

FP32 = mybir.dt.float32
BF16 = mybir.dt.bfloat16
NPBF16 = ml_dtypes.bfloat16


def prep_inputs(in1: np.ndarray, in2: np.ndarray) -> list[dict]:
    """Host-side prep: tile layout + 1/256 prescale for in1, zero-pad for
    in2, both cast to bf16.  Returns per-core input maps."""
    # [B, C, H, W] -> [B, C, yb, x, y32] -> flat [B, C, yb, 4096], col x*32+y
    in1p = (in1.reshape(B, C, N_YB, YB, W).swapaxes(3, 4)
            * np.float32(1.0 / C)).astype(NPBF16)
    in1p = np.ascontiguousarray(in1p.reshape(B, C, N_YB, W * YB))
    in2p = np.zeros((B, C, HP, WP), dtype=NPBF16)
    in2p[:, :, PAD:PAD + H, PAD:PAD + W] = in2.astype(NPBF16)
    # constant scatter-index table for the local_scatter de-shear:
    # data (per pair) is sv [128, 2, 40, 12] flat; slot (h, r, c) of pixel
    # p = 32g + u maps to output h*81 + (r-u)*9 + (c-g) when in-band, else -1
    u = (np.arange(128) % 32)[:, None, None, None]
    g = (np.arange(128) // 32)[:, None, None, None]
    h = np.arange(2)[None, :, None, None]
    r = np.arange(MV_Y)[None, None, :, None]
    c = np.arange(MV_X)[None, None, None, :]
    dy, dx = r - u, c - g
    cidx = np.where(
        (dy >= 0) & (dy < D) & (dx >= 0) & (dx < D),
        h * CH + dy * D + dx, -1,
    ).astype(np.int16).reshape(128, 2 * MV_Y * MV_X)
    return [{"in1": in1p[b], "in2": in2p[b], "cidx": cidx} for b in range(B)]


def build_nc():
    nc = bacc.Bacc("TRN2", target_bir_lowering=False, debug=False)
    in1_t = nc.dram_tensor("in1", [C, N_YB, W * YB], BF16, kind="ExternalInput")
    in2_t = nc.dram_tensor("in2", [C, HP, WP], BF16, kind="ExternalInput")
    out_d = nc.dram_tensor("out", [CH, H, W], BF16, kind="ExternalOutput").ap()
    cidx_t = nc.dram_tensor(
        "cidx", [128, 2 * MV_Y * MV_X], mybir.dt.int16, kind="ExternalInput")
    # scratch row pitch 369 (= WIN + D) and per-pixel block pitch
    # 16*369 - 9 = 5895: row (p, kb) lives at p*5895 + kb*369.  The gather
    # for pixel p reads [9u, 9u+81) of each row, so its (u, kb) dims have
    # strides 5904 = 16*369 and 369 -> they merge into one 512-count dim,
    # keeping the gather AP at 3 dims.  The 9-elem row slack overlaps the
    # NEXT pixel's first row but is never written (rows are 360 long).
    RPITCH = WIN + D            # 369
    KBS = [kb for b in YB_BATCHES for kb in b]

    def ppitch(kbn):
        return kbn * RPITCH - D

    sd_t = [
        nc.dram_tensor(
            f"sd{j}",
            [127 * ppitch(kbn) + (kbn - 1) * RPITCH + WIN],
            BF16, kind="Internal",
        ) if not SCATTER[j] else None
        for j, kbn in enumerate(KBS)
    ]

    # element strides of the dram input layouts
    S1_C, S1_YB = N_YB * W * YB, W * YB          # in1 [C, 4, 4096]
    S2_C = HP * WP                               # in2 [C, 136, 136]

    with tile.TileContext(nc) as tc, ExitStack() as es:
        const_pool = es.enter_context(tc.tile_pool(name="const", bufs=1))
        in1_pool = es.enter_context(tc.tile_pool(name="in1p", bufs=4))
        in2_pool = es.enter_context(tc.tile_pool(name="in2p", bufs=3))
        wv_pool = es.enter_context(tc.tile_pool(name="wv", bufs=3))
        sv_pool = es.enter_context(tc.tile_pool(name="sv", bufs=6))
        tg_pool = es.enter_context(tc.tile_pool(name="tg", bufs=3))
        o_pool = es.enter_context(tc.tile_pool(name="oasm", bufs=2))
        ps_pool = es.enter_context(tc.tile_pool(name="ps", bufs=3, space="PSUM"))
        ps2_pool = es.enter_context(tc.tile_pool(name="ps2", bufs=2, space="PSUM"))

        # ---- identity matrix (bf16) for TensorE transpose ----
        ones = const_pool.tile([128, 128], FP32, tag="ones")
        identf = const_pool.tile([128, 128], FP32, tag="identf")
        ident = const_pool.tile([128, 128], BF16, tag="ident")
        nc.gpsimd.memset(ones[:, :], 1.0)
        # iota[p, f] = f - p; ident = where(iota == 0, ones, 0)
        nc.gpsimd.affine_select(
            identf[:, :], ones[:, :], pattern=[[1, 128]],
            compare_op=mybir.AluOpType.is_equal, fill=0.0,
            base=0, channel_multiplier=-1,
        )
        nc.vector.tensor_copy(ident[:, :], identf[:, :])
        cidx = const_pool.tile([128, 2 * MV_Y * MV_X], mybir.dt.int16, tag="cidx")
        nc.sync.dma_start(cidx[:, :], cidx_t.ap())

        # ---- per-y-block input tiles (bufs=2 pools stagger the loads:
        # yb+2's load waits on yb's last consumer via buffer reuse, so dumps
        # and gathers interleave with loads on the DMA engines) ----
        ybtiles = {}

        def issue_loads(yb):
            in1t = in1_pool.tile([128, 2, W * YB], BF16, tag="in1t")
            in2t = in2_pool.tile([128, 2, MV_Y, WP], BF16, tag="in2t")
            # rows [0, 8) of this window = rows [32, 40) of the previous one:
            # copy them SBUF->SBUF on idle gpsimd instead of re-reading HBM
            r0 = 0 if yb == 0 else 8
            if yb > 0:
                prev = ybtiles[yb - 1][1]
                nc.gpsimd.tensor_copy(in2t[:, :, 0:8, :], prev[:, :, YB:MV_Y, :])
            # yb0 primes the pipeline: split its loads into column halves so
            # the first matmuls start ~5us earlier
            halves = ((0, 2048, 0, 76), (2048, 4096, 76, WP)) if yb == 0                 else ((0, 4096, 0, WP),)
            for c0, c1, w0, w1 in halves:
                for cb in range(2):
                    nc.sync.dma_start(
                        in1t[:, cb, c0:c1],
                        bass.AP(in1_t, cb * 128 * S1_C + yb * S1_YB + c0,
                                [[S1_C, 128], [1, c1 - c0]]),
                    )
                    nc.sync.dma_start(
                        in2t[:, cb, r0:MV_Y, w0:w1],
                        bass.AP(in2_t,
                                cb * 128 * S2_C + (yb * YB + r0) * WP + w0,
                                [[S2_C, 128], [WP, MV_Y - r0], [1, w1 - w0]]),
                    )
            ybtiles[yb] = (in1t, in2t)

        issue_loads(0)

        # software-pipelined phase 2: emitted one batch late so the next
        # batch's matmuls are never program-ordered behind this batch's
        # dump -> gather chain
        pending = []

        def phase2(j, KBN, PPITCH, xb_base, oasm, tg):
            if tg is None:
                tg = tg_pool.tile([128, KBN, CH], BF16, tag="tg")
                # elem offset for (g, u, kb, k):
                #   g*32*PPITCH + u*(PPITCH+9) + kb*369 + k ((u, kb) merged)
                nc.sync.dma_start(
                    tg[:, :, :],
                    bass.AP(sd_t[j], 0,
                            [[32 * PPITCH, NG], [RPITCH, 32 * KBN], [1, CH]]),
                )
            # 4 transposes share one PSUM bank, then one merged evacuate
            for kq in range(KBN // 4):
                ps2 = ps2_pool.tile([128, 4, XBW, YB], BF16, tag="ps2")
                for kk in range(4):
                    kb = 4 * kq + kk
                    nc.tensor.transpose(
                        ps2[0:CH, kk, :, :], tg[:, kb, :], ident[:, :]
                    )
                # evacuate with (kq,x-outer,y-inner) -> (y, x) reorder
                x0 = (xb_base + 4 * kq) * XBW
                dst = oasm[0:CH, :, x0:x0 + 4 * XBW].rearrange(
                    "p y (kk x) -> p y kk x", kk=4
                ).transpose([0, 2, 3, 1])
                src = ps2[0:CH, :, :, :]
                if kq % 2 == 0:
                    nc.vector.tensor_copy(dst, src)
                else:
                    nc.scalar.copy(dst, src)

        def flush_pending():
            while pending:
                args, out_yb = pending.pop(0)
                phase2(*args)
                if out_yb is not None:
                    yb_, oasm_ = out_yb
                    nc.gpsimd.dma_start(
                        out_d[:, yb_ * YB:(yb_ + 1) * YB, :], oasm_[0:CH, :, :]
                    )

        for yb in range(N_YB):
            if yb + 1 < N_YB:
                issue_loads(yb + 1)
            in1t, in2t = ybtiles[yb]
            oasm = o_pool.tile([128, YB, W], BF16, tag="oasm")
            xb_base = 0
            for bj, KBN in enumerate(YB_BATCHES[yb]):
                j = sum(len(b) for b in YB_BATCHES[:yb]) + bj
                PPITCH = ppitch(KBN)
                scat = SCATTER[j]
                if scat:
                    wv = None
                    tg = tg_pool.tile([128, KBN, CH], BF16, tag="tg")
                else:
                    wv = wv_pool.tile([128, KBN, MV_Y, D], BF16, tag="wv")
                    tg = None
                # ---------------- phase 1: KBN tiles, in pairs ----------------
                # two tiles share one bank-aligned PSUM pair-tile so a single
                # evacuate (cast to bf16) amortizes the PSUM-access fixed cost;
                # then either (a) window-compact SBUF->SBUF on DVE (all-bf16
                # packed operands hit the 4x perf mode) + DRAM bounce, or
                # (b) de-shear the pair directly on gpsimd via local_scatter
                for kp in range(KBN // 2):
                    ps = ps_pool.tile([128, 2, 512], FP32, tag="ps")
                    for i in range(2):
                        kb = 2 * kp + i
                        xb = xb_base + kb
                        x0 = xb * XBW
                        pso = ps[:, i, 0:MV_Y * MV_X].rearrange(
                            "p (a b) -> p a b", a=MV_Y
                        )
                        for cb in range(2):
                            nc.tensor.matmul(
                                pso,
                                in1t[:, cb, xb * 128:(xb + 1) * 128],
                                in2t[:, cb, :, x0:x0 + MV_X],
                                start=(cb == 0),
                                stop=(cb == 1),
                            )
                    sv = sv_pool.tile([128, 2, MV_Y, MV_X], BF16, tag="sv")
                    sv_src = ps[:, :, 0:MV_Y * MV_X].rearrange(
                        "p c (a b) -> p c a b", a=MV_Y
                    )
                    if kp % 4 == 3:
                        nc.vector.tensor_copy(sv[:, :, :, :], sv_src)
                    else:
                        nc.scalar.copy(sv[:, :, :, :], sv_src)
                    if scat:
                        nc.gpsimd.local_scatter(
                            tg[:, 2 * kp:2 * kp + 2, :].rearrange(
                                "p a b -> p (a b)"),
                            sv[:, :, :, :].rearrange("p a b c -> p (a b c)"),
                            cidx[:, :],
                            128, 2 * CH, 2 * MV_Y * MV_X,
                        )
                    else:
                        for i in range(2):
                            kb = 2 * kp + i
                            for g in range(NG):
                                src = sv[32 * g:32 * (g + 1), i, :, g:g + D]
                                dst = wv[32 * g:32 * (g + 1), kb, :, :]
                                nc.vector.tensor_copy(dst, src)
                if not scat:
                    # batch dump: row (p, kb) at p*PPITCH + kb*369, 360 elems
                    nc.sync.dma_start(
                        bass.AP(sd_t[j], 0,
                                [[PPITCH, 128], [RPITCH, KBN], [1, WIN]]),
                        wv[:, :, :, :],
                    )
                flush_pending()
                last = bj == len(YB_BATCHES[yb]) - 1
                pending.append(
                    ((j, KBN, PPITCH, xb_base, oasm, tg),
                     (yb, oasm) if last else None)
                )
                xb_base += KBN
        flush_pending()

    nc.compile()
    return nc


_NC_CACHE = None


def _get_nc():
    global _NC_CACHE
    if _NC_CACHE is None:
        _NC_CACHE = build_nc()
    return _NC_CACHE


def kernel(in1: np.ndarray, in2: np.ndarray) -> np.ndarray:
    nc = _get_nc()
    in1 = np.asarray(in1, dtype=np.float32)
    in2 = np.asarray(in2, dtype=np.float32)
    assert in1.shape == (B, C, H, W) and in2.shape == (B, C, H, W)
    in_maps = prep_inputs(in1, in2)
    res = bass_utils.run_bass_kernel_spmd(nc, in_maps, core_ids=list(range(B)))
    out = np.stack([res.results[b]["out"] for b in range(B)], axis=0)
    return out.astype(np.float32)
